# revision 27
# baseline (speedup 1.0000x reference)
"""Trainium2 Bass kernel for nn_DetectorKmeans (retrieval_knn).

density[n] = sum_k (pr[k]*var[k]) / ||X[n]-C[k]||^2  - threshold

Data-parallel over 8 NeuronCores (X sharded along N). Per core:
  * Augmented bf16 matmul produces PSUM T[n-tile, k-half] = full squared
    distance directly: main rows contract X^T against -2*C^T; four extra
    contraction rows carry x_sq (hi/lo bf16 pair) and c_sq (hi/lo), so
    T = x_sq - 2<x,c> + c_sq at ~1e-4 relative accuracy.
  * X is pre-transposed to [D, R] bf16 on the host so the contraction dim
    lies on SBUF partitions with contiguous DMA; X^T tiles are the
    stationary operand (each weight load feeds both k-halves).
  * The 4 augmented matmuls per k-half run concurrently in disjoint PE
    row-groups via tile_position.
  * ACT-engine reciprocal (measured ~1e-5 rel err on HW) converts T to
    1/sqdist; DVE tensor_tensor_reduce multiplies by w = pr*var and
    accumulates over k in one pass; a final scalar_tensor_tensor fuses
    the k-half combine with the threshold subtraction.
"""

import numpy as np
import ml_dtypes

BF16 = ml_dtypes.bfloat16

N, K, D = 65536, 1024, 512
NCORES = 8
R = N // NCORES
F = 512  # rows per supertile
KH = 512  # k-half
NSUP = R // F

_NC = None


def _act_recip(nc, mybir, out, in_, accum=None):
    """ACT-engine reciprocal (bypasses the library guard; measured max rel
    err ~1.2e-5 on TRN2 HW for this kernel's value range). With accum, the
    free-axis sum of the (pre-rounding, f32) reciprocals lands in accum."""
    dt = mybir.dt
    eng = nc.scalar
    ins = [
        eng.lower_ap(in_),
        mybir.ImmediateValue(dtype=dt.float32, value=0.0),
        mybir.ImmediateValue(dtype=dt.float32, value=1.0),
        mybir.ImmediateValue(dtype=dt.float32, value=0.0),
    ]
    outs = [eng.lower_ap(out)]
    if accum is not None:
        outs.append(eng.lower_ap(accum))
    return eng.add_instruction(
        mybir.InstActivation(
            name=nc.get_next_instruction_name(),
            func=mybir.ActivationFunctionType.Reciprocal,
            ins=ins,
            outs=outs,
        )
    )


def _build_nc(r=R, num_devices=NCORES, pack_aug=True, out_dma_strided=True):
    import concourse.bacc as bacc
    import concourse.tile as tile
    import concourse.mybir as mybir

    dt = mybir.dt
    nsup = r // F
    nc = bacc.Bacc(
        "TRN2", target_bir_lowering=False, debug=False, num_devices=num_devices
    )
    xt_d = nc.dram_tensor("xt", [D, r], dt.bfloat16, kind="ExternalInput")
    arx_d = nc.dram_tensor("arx", [5, r], dt.bfloat16, kind="ExternalInput")
    cm_d = nc.dram_tensor("cm", [D, K], dt.bfloat16, kind="ExternalInput")
    carq_d = nc.dram_tensor("carq", [128, 2 * KH], dt.bfloat16, kind="ExternalInput")
    th_d = nc.dram_tensor("th", [128, 1], dt.float32, kind="ExternalInput")
    out_d = nc.dram_tensor("out", [r], dt.float32, kind="ExternalOutput")

    with tile.TileContext(nc) as tc:
        with (
            tc.tile_pool(name="const", bufs=1) as constp,
            tc.tile_pool(name="xin", bufs=3) as xinp,
            tc.tile_pool(name="rec", bufs=4) as recp,
            tc.tile_pool(name="accp", bufs=10) as accp,
            tc.tile_pool(name="osb", bufs=2) as osbp,
            tc.tile_pool(name="psT", bufs=4, space="PSUM") as psT,
        ):
            cm = constp.tile([128, 4, K], dt.bfloat16)
            nc.sync.dma_start(cm[:], cm_d.rearrange("(c p) k -> p c k", p=128))
            carq = constp.tile([128, 2, KH], dt.bfloat16)
            nc.sync.dma_start(carq[:], carq_d.rearrange("p (h k) -> p h k", h=2))
            th = constp.tile([128, 1], dt.float32)
            nc.sync.dma_start(th[:], th_d[:])

            xt_r = xt_d.rearrange("(c p) n -> p c n", p=128)
            for s in range(nsup):
                n0 = s * F
                xt = xinp.tile([128, 4, F], dt.bfloat16, tag="xt")
                nc.sync.dma_start(xt[:], xt_r[:, :, n0 : n0 + F])
                if pack_aug:
                    augl = xinp.tile([128, 128], dt.bfloat16, tag="augl")
                    for g in range(4):
                        nc.sync.dma_start(
                            augl[32 * g : 32 * g + 5, :],
                            arx_d[:, n0 + 128 * g : n0 + 128 * (g + 1)],
                        )
                else:
                    augl = xinp.tile([5, F], dt.bfloat16, tag="augl")
                    nc.sync.dma_start(augl[:], arx_d[:, n0 : n0 + F])

                Ts = {}
                for t in range(4):
                    # [128, 1024] spans two PSUM banks; each matmul's output
                    # slice stays within one bank.
                    Ts[t] = psT.tile([128, K], dt.float32, tag="T", name=f"T{t}")
                    for i in range(4):
                        lhs = xt[:, i, 128 * t : 128 * (t + 1)]
                        for h in range(2):
                            nc.tensor.matmul(
                                Ts[t][:, KH * h : KH * (h + 1)],
                                lhs,
                                cm[:, i, KH * h : KH * (h + 1)],
                                start=(i == 0),
                                stop=False,
                            )
                # augmented rows: 4 row-tiles packed into disjoint PE row-groups
                for h in range(2):
                    for t in range(4):
                        if pack_aug:
                            nc.tensor.matmul(
                                Ts[t][:, KH * h : KH * (h + 1)],
                                augl[32 * t : 32 * t + 5, :],
                                carq[32 * t : 32 * t + 5, h, :],
                                start=False,
                                stop=True,
                                tile_position=(32 * t, 0),
                            )
                        else:
                            nc.tensor.matmul(
                                Ts[t][:, KH * h : KH * (h + 1)],
                                augl[:, 128 * t : 128 * (t + 1)],
                                carq[0:5, h, :],
                                start=False,
                                stop=True,
                            )
                outsb = osbp.tile([128, 4], dt.float32, tag="outsb")
                for t in range(4):
                    rr = recp.tile([128, K], dt.bfloat16, tag="r", name="rr")
                    acc = accp.tile([128, 1], dt.float32, tag="acc", name="acc")
                    # rr (dummy) = w/s; acc = sum_k w/s (f32 internal)
                    _act_recip(nc, mybir, rr[:], Ts[t][:], accum=acc[:])
                    nc.vector.tensor_scalar_sub(
                        outsb[:, t : t + 1], acc[:], th[:]
                    )
                if out_dma_strided:
                    nc.sync.dma_start(
                        out_d[n0 : n0 + F].rearrange("(a p) -> p a", p=128),
                        outsb[:],
                    )
                else:
                    for t in range(4):
                        nc.sync.dma_start(
                            out_d[n0 + 128 * t : n0 + 128 * (t + 1)].rearrange(
                                "(a p) -> p a", p=128
                            ),
                            outsb[:, t : t + 1],
                        )
    nc.compile()
    return nc


def _host_prep_shared(center, var, pr, threshold):
    C32 = center.astype(np.float64)
    w = pr.astype(np.float64) * var.astype(np.float64)
    invw = 1.0 / w
    # cm[d,k] = bf16(-2 * C[k,d] / w[k]) -> PSUM T = sqdist / w directly,
    # so ACT reciprocal emits w/sqdist and its accum is the density sum.
    cm = np.ascontiguousarray((-2.0 * C32 * invw[:, None]).T).astype(BF16)  # [D, K]
    # consistent csq/w from the rounded cm: the effective center is
    # c_hat = -cm*w/2, so csq/w = (w/4) * sum_d cm^2
    cmf = cm.astype(np.float64)
    csqw = (w / 4.0 * (cmf**2).sum(0)).astype(np.float32)
    csqw_hi = csqw.astype(BF16)
    csqw_lo = (csqw - csqw_hi.astype(np.float32)).astype(BF16)
    invw32 = invw.astype(np.float32)
    invw_hi = invw32.astype(BF16)
    invw_lo = (invw32 - invw_hi.astype(np.float32)).astype(BF16)
    # aug rhs rows (pair with lhsT rows [xsq_hi, xsq_hi, xsq_lo, 1, 1]):
    aug_rows = np.stack([invw_hi, invw_lo, invw_hi, csqw_hi, csqw_lo])  # [5, K]
    carq = np.zeros((128, 2 * KH), BF16)
    for g in range(4):
        for rrow in range(5):
            carq[32 * g + rrow, :] = aug_rows[rrow, :]
    th = np.full((128, 1), np.float32(np.asarray(threshold).reshape(-1)[0]))
    return cm, carq, th


def _host_prep_shard(Xs):
    Xb = Xs.astype(BF16)
    xt = np.ascontiguousarray(Xb.T)
    xsq = (Xb.astype(np.float32) ** 2).sum(1, dtype=np.float64).astype(np.float32)
    xsq_hi = xsq.astype(BF16)
    xsq_lo = (xsq - xsq_hi.astype(np.float32)).astype(BF16)
    onesr = np.ones(Xs.shape[0], BF16)
    arx = np.ascontiguousarray(np.stack([xsq_hi, xsq_hi, xsq_lo, onesr, onesr]))
    return xt, arx


def kernel(X, center, var, pr, threshold):
    global _NC
    X = np.asarray(X)
    cm, carq, th = _host_prep_shared(
        np.asarray(center), np.asarray(var), np.asarray(pr), np.asarray(threshold)
    )
    in_maps = []
    for c in range(NCORES):
        xt, arx = _host_prep_shard(X[c * R : (c + 1) * R])
        in_maps.append(dict(xt=xt, arx=arx, cm=cm, carq=carq, th=th))

    if _NC is None:
        _NC = _build_nc()

    from concourse.bass_utils import run_bass_kernel_spmd

    res = run_bass_kernel_spmd(_NC, in_maps, core_ids=list(range(NCORES)))
    out = np.concatenate([res.results[c]["out"] for c in range(NCORES)])
    return np.ascontiguousarray(out, dtype=np.float32)


# revision 29
# speedup vs baseline: 1.3212x; 1.3212x over previous
"""Trainium2 Bass kernel for nn_DetectorKmeans (retrieval_knn).

density[n] = sum_k (pr[k]*var[k]) / ||X[n]-C[k]||^2  - threshold

Data-parallel over 8 NeuronCores (X sharded along N). Per core:
  * Augmented bf16 matmul produces PSUM T[n-tile, k-half] = full squared
    distance directly: main rows contract X^T against -2*C^T; four extra
    contraction rows carry x_sq (hi/lo bf16 pair) and c_sq (hi/lo), so
    T = x_sq - 2<x,c> + c_sq at ~1e-4 relative accuracy.
  * X is pre-transposed to [D, R] bf16 on the host so the contraction dim
    lies on SBUF partitions with contiguous DMA; X^T tiles are the
    stationary operand (each weight load feeds both k-halves).
  * The 4 augmented matmuls per k-half run concurrently in disjoint PE
    row-groups via tile_position.
  * ACT-engine reciprocal (measured ~1e-5 rel err on HW) converts T to
    1/sqdist; DVE tensor_tensor_reduce multiplies by w = pr*var and
    accumulates over k in one pass; a final scalar_tensor_tensor fuses
    the k-half combine with the threshold subtraction.
"""

import numpy as np
import ml_dtypes

BF16 = ml_dtypes.bfloat16

N, K, D = 65536, 1024, 512
NCORES = 8
R = N // NCORES
F = 512  # rows per supertile
KH = 512  # k-half
NSUP = R // F

_NC = None


def _act_recip(nc, mybir, out, in_, accum=None):
    """ACT-engine reciprocal (bypasses the library guard; measured max rel
    err ~1.2e-5 on TRN2 HW for this kernel's value range). With accum, the
    free-axis sum of the (pre-rounding, f32) reciprocals lands in accum."""
    dt = mybir.dt
    eng = nc.scalar
    ins = [
        eng.lower_ap(in_),
        mybir.ImmediateValue(dtype=dt.float32, value=0.0),
        mybir.ImmediateValue(dtype=dt.float32, value=1.0),
        mybir.ImmediateValue(dtype=dt.float32, value=0.0),
    ]
    outs = [eng.lower_ap(out)]
    if accum is not None:
        outs.append(eng.lower_ap(accum))
    return eng.add_instruction(
        mybir.InstActivation(
            name=nc.get_next_instruction_name(),
            func=mybir.ActivationFunctionType.Reciprocal,
            ins=ins,
            outs=outs,
        )
    )


def _build_nc(r=R, num_devices=NCORES, pack_aug=True, out_dma_strided=True):
    import concourse.bacc as bacc
    import concourse.tile as tile
    import concourse.mybir as mybir

    dt = mybir.dt
    nsup = r // F
    nc = bacc.Bacc(
        "TRN2", target_bir_lowering=False, debug=False, num_devices=num_devices
    )
    xt_d = nc.dram_tensor("xt", [D, r], dt.bfloat16, kind="ExternalInput")
    arx_d = nc.dram_tensor("arx", [5, r], dt.bfloat16, kind="ExternalInput")
    cm_d = nc.dram_tensor("cm", [D, K], dt.bfloat16, kind="ExternalInput")
    carq_d = nc.dram_tensor("carq", [128, 2 * KH], dt.bfloat16, kind="ExternalInput")
    th_d = nc.dram_tensor("th", [128, 1], dt.float32, kind="ExternalInput")
    out_d = nc.dram_tensor("out", [r], dt.float32, kind="ExternalOutput")

    with tile.TileContext(nc) as tc:
        with (
            tc.tile_pool(name="const", bufs=1) as constp,
            tc.tile_pool(name="xin", bufs=3) as xinp,
            tc.tile_pool(name="rec", bufs=4) as recp,
            tc.tile_pool(name="accp", bufs=10) as accp,
            tc.tile_pool(name="osb", bufs=2) as osbp,
            tc.tile_pool(name="psT", bufs=4, space="PSUM") as psT,
        ):
            cm = constp.tile([128, 4, K], dt.bfloat16)
            cm_r = cm_d.rearrange("(c p) k -> p c k", p=128)
            for i in range(4):
                nc.sync.dma_start(cm[:, i, :], cm_r[:, i, :])
            carq = constp.tile([128, 2, KH], dt.bfloat16)
            nc.sync.dma_start(carq[:], carq_d.rearrange("p (h k) -> p h k", h=2))
            th = constp.tile([128, 1], dt.float32)
            nc.sync.dma_start(th[:], th_d[:])

            xt_r = xt_d.rearrange("(c p) n -> p c n", p=128)
            for s in range(nsup):
                n0 = s * F
                xt = xinp.tile([128, 4, F], dt.bfloat16, tag="xt")
                nc.sync.dma_start(xt[:], xt_r[:, :, n0 : n0 + F])
                if pack_aug:
                    augl = xinp.tile([128, 128], dt.bfloat16, tag="augl")
                    for g in range(4):
                        nc.sync.dma_start(
                            augl[32 * g : 32 * g + 5, :],
                            arx_d[:, n0 + 128 * g : n0 + 128 * (g + 1)],
                        )
                else:
                    augl = xinp.tile([5, F], dt.bfloat16, tag="augl")
                    nc.sync.dma_start(augl[:], arx_d[:, n0 : n0 + F])

                Ts = {}
                for t in range(4):
                    # [128, 1024] spans two PSUM banks; each matmul's output
                    # slice stays within one bank.
                    Ts[t] = psT.tile([128, K], dt.float32, tag="T", name=f"T{t}")
                # augmented rows first (they open each accumulation group), so
                # each row-tile's reciprocal can fire as soon as its own main
                # matmuls finish. 4 row-tiles packed into disjoint PE row-groups.
                for h in range(2):
                    for t in range(4):
                        if pack_aug:
                            nc.tensor.matmul(
                                Ts[t][:, KH * h : KH * (h + 1)],
                                augl[32 * t : 32 * t + 5, :],
                                carq[32 * t : 32 * t + 5, h, :],
                                start=True,
                                stop=False,
                                tile_position=(32 * t, 0),
                            )
                        else:
                            nc.tensor.matmul(
                                Ts[t][:, KH * h : KH * (h + 1)],
                                augl[:, 128 * t : 128 * (t + 1)],
                                carq[0:5, h, :],
                                start=True,
                                stop=False,
                            )
                outsb = osbp.tile([128, 4], dt.float32, tag="outsb")
                for t in range(4):
                    for i in range(4):
                        lhs = xt[:, i, 128 * t : 128 * (t + 1)]
                        for h in range(2):
                            nc.tensor.matmul(
                                Ts[t][:, KH * h : KH * (h + 1)],
                                lhs,
                                cm[:, i, KH * h : KH * (h + 1)],
                                start=False,
                                stop=(i == 3),
                            )
                    rr = recp.tile([128, K], dt.bfloat16, tag="r", name="rr")
                    acc = accp.tile([128, 1], dt.float32, tag="acc", name="acc")
                    # rr (dummy) = w/s; acc = sum_k w/s (f32 internal)
                    _act_recip(nc, mybir, rr[:], Ts[t][:], accum=acc[:])
                    nc.vector.tensor_scalar_sub(
                        outsb[:, t : t + 1], acc[:], th[:]
                    )
                if out_dma_strided:
                    nc.sync.dma_start(
                        out_d[n0 : n0 + F].rearrange("(a p) -> p a", p=128),
                        outsb[:],
                    )
                else:
                    for t in range(4):
                        nc.sync.dma_start(
                            out_d[n0 + 128 * t : n0 + 128 * (t + 1)].rearrange(
                                "(a p) -> p a", p=128
                            ),
                            outsb[:, t : t + 1],
                        )
    nc.compile()
    return nc


def _host_prep_shared(center, var, pr, threshold):
    C32 = center.astype(np.float64)
    w = pr.astype(np.float64) * var.astype(np.float64)
    invw = 1.0 / w
    # cm[d,k] = bf16(-2 * C[k,d] / w[k]) -> PSUM T = sqdist / w directly,
    # so ACT reciprocal emits w/sqdist and its accum is the density sum.
    cm = np.ascontiguousarray((-2.0 * C32 * invw[:, None]).T).astype(BF16)  # [D, K]
    # consistent csq/w from the rounded cm: the effective center is
    # c_hat = -cm*w/2, so csq/w = (w/4) * sum_d cm^2
    cmf = cm.astype(np.float64)
    csqw = (w / 4.0 * (cmf**2).sum(0)).astype(np.float32)
    csqw_hi = csqw.astype(BF16)
    csqw_lo = (csqw - csqw_hi.astype(np.float32)).astype(BF16)
    invw32 = invw.astype(np.float32)
    invw_hi = invw32.astype(BF16)
    invw_lo = (invw32 - invw_hi.astype(np.float32)).astype(BF16)
    # aug rhs rows (pair with lhsT rows [xsq_hi, xsq_hi, xsq_lo, 1, 1]):
    aug_rows = np.stack([invw_hi, invw_lo, invw_hi, csqw_hi, csqw_lo])  # [5, K]
    carq = np.zeros((128, 2 * KH), BF16)
    for g in range(4):
        for rrow in range(5):
            carq[32 * g + rrow, :] = aug_rows[rrow, :]
    th = np.full((128, 1), np.float32(np.asarray(threshold).reshape(-1)[0]))
    return cm, carq, th


def _host_prep_shard(Xs):
    Xb = Xs.astype(BF16)
    xt = np.ascontiguousarray(Xb.T)
    xsq = (Xb.astype(np.float32) ** 2).sum(1, dtype=np.float64).astype(np.float32)
    xsq_hi = xsq.astype(BF16)
    xsq_lo = (xsq - xsq_hi.astype(np.float32)).astype(BF16)
    onesr = np.ones(Xs.shape[0], BF16)
    arx = np.ascontiguousarray(np.stack([xsq_hi, xsq_hi, xsq_lo, onesr, onesr]))
    return xt, arx


def kernel(X, center, var, pr, threshold):
    global _NC
    X = np.asarray(X)
    cm, carq, th = _host_prep_shared(
        np.asarray(center), np.asarray(var), np.asarray(pr), np.asarray(threshold)
    )
    in_maps = []
    for c in range(NCORES):
        xt, arx = _host_prep_shard(X[c * R : (c + 1) * R])
        in_maps.append(dict(xt=xt, arx=arx, cm=cm, carq=carq, th=th))

    if _NC is None:
        _NC = _build_nc()

    from concourse.bass_utils import run_bass_kernel_spmd

    res = run_bass_kernel_spmd(_NC, in_maps, core_ids=list(range(NCORES)))
    out = np.concatenate([res.results[c]["out"] for c in range(NCORES)])
    return np.ascontiguousarray(out, dtype=np.float32)
